# revision 63
# baseline (speedup 1.0000x reference)
"""Trainium2 Bass kernel for nn_DualChannelTransformer.

Sharding: 8 cores = 4 batches x 2 channels. Each core runs one channel's
stack; cross-attention K/V activations are exchanged pairwise (fp8 wire)
per layer, pipelined in two S-halves behind the FFN+LN. Layer-0 partner
activations are computed locally from the (tiny) raw wrist inputs, so no
startup collective.

fp8 pipeline: all projection / FFN weights are stored as fp8e4 hi+lo
residual pairs pre-scaled by 64 (keeps the lo part out of the subnormal
range; combined quantization error ~0.18% which is bf16-level). Matmuls
run in DoubleRow perf mode (K=256 per instruction at 0.5 cycles/column =
2x bf16). Activations are quantized to fp8 at 1/8 scale (x8 = x/8), so
weight(64) x act(1/8) products come out at 8x natural scale and evictions
fold the 1/8 with the existing scalar ops. q/k/v/ctx/probs live at
natural scale; the softmax denominator ones-column is set to 8.0 so the
reciprocal also divides the ctx eviction by 8 (the O-projection expects
ctx/8). Scores stay plain fp8 (K=64 contraction, 1 cyc/col). The V and K
biases are folded out exactly (V bias into the O bias via b_v @ W_o;
K bias cancels in softmax), LayerNorm emits bf16 (residual stream) plus
an fp8/8 copy for the matmul operands.
"""

import os
import sys

import numpy as np

for _p in ("/opt/trn_rl_repo", "/root/.axon_site/_ro/trn_rl_repo"):
    if os.path.isdir(_p) and _p not in sys.path:
        sys.path.insert(0, _p)

import ml_dtypes

import concourse.bass as bass
import concourse.tile as tile
from concourse import bacc, mybir
from concourse.bass import ds
from concourse.bass_utils import run_bass_kernel_spmd

F32 = mybir.dt.float32
F32R = mybir.dt.float32r
BF16 = mybir.dt.bfloat16
F8 = mybir.dt.float8e4
AF = mybir.ActivationFunctionType
OP = mybir.AluOpType
DR = mybir.MatmulPerfMode.DoubleRow
BF = ml_dtypes.bfloat16
F8NP = ml_dtypes.float8_e4m3

B, S, IN, D, H, LAYERS, F = 4, 1024, 6, 512, 8, 4, 2048
DK = D // H
EPS = 1e-5
NCORES = 8
DC = D // 128   # 4 activation partition chunks
FC = F // 128   # 16
NQ = S // 512   # 2 moving-dim chunks
NST = S // 128  # 8 k tiles
WS = 64.0       # weight hi/lo scale
XS = 0.125      # activation storage scale (x8 = x/8)
EB = -1.0       # exp bias (headroom against fp8 overflow; cancels in ratio)

_CACHE = {}


class PsumHalves:
    """Hands out [128,512] PSUM half-tiles packed two-per-bank-pair."""

    def __init__(self, pbig):
        self.pbig = pbig
        self.cur = None
        self.par = 0

    def next(self):
        if self.par == 0:
            self.cur = self.pbig.tile([128, 1024], F32, tag="big",
                                      name="ph")
        sl = self.cur[:, self.par * 512:(self.par + 1) * 512]
        self.par ^= 1
        return sl



def _emit_ln_half(nc, pools, a_t, g_sb, b_sb, ln_i, nq, out, out8,
                  mean_out=None, post_half=None):
    """One S-half of a LayerNorm over D (partition axis) of a_t."""
    act, pbig, consts = pools["act"], pools["pbig"], pools["consts"]
    oavg_bf = consts["oavg_bf"]
    eps_col = consts["eps_col"]
    s0 = nq * 512
    sq = act.tile([128, 4, 512], BF16, tag="sq", bufs=2, name="sqh")
    for kc in range(DC):
        eng = nc.vector if kc % 2 == 0 else nc.gpsimd
        eng.tensor_tensor(out=sq[:, kc, :],
                          in0=a_t[:, kc, s0:s0 + 512],
                          in1=a_t[:, kc, s0:s0 + 512], op=OP.mult)
    mps = pbig.tile([128, 1024], F32, tag="big", name="lnmps")
    for kc in range(DC):
        nc.tensor.matmul(mps[:, 0:512], oavg_bf[:],
                         a_t[:, kc, s0:s0 + 512],
                         start=(kc == 0), stop=(kc == DC - 1))
    for kc in range(DC):
        nc.tensor.matmul(mps[:, 512:1024], oavg_bf[:], sq[:, kc, :],
                         start=(kc == 0), stop=(kc == DC - 1))
    mean_sb = act.tile([128, 512], BF16, tag="mnb", bufs=2, name="lnmean")
    nc.vector.tensor_copy(out=mean_sb[:], in_=mps[:, 0:512])
    m2 = act.tile([128, 512], F32, tag="lnt", bufs=2, name="lnm2")
    nc.vector.tensor_tensor(out=m2[:], in0=mean_sb[:], in1=mean_sb[:],
                            op=OP.mult)
    work = act.tile([128, 512], F32, tag="lnt2", bufs=2, name="lnwork")
    nc.vector.tensor_tensor(out=work[:], in0=mps[:, 512:1024], in1=m2[:],
                            op=OP.subtract)
    nc.scalar.activation(out=work[:], in_=work[:], func=AF.Sqrt,
                         bias=eps_col[:])
    rstd = act.tile([128, 512], BF16, tag="rstd", bufs=2, name="lnrstd")
    nc.vector.reciprocal(out=rstd[:], in_=work[:])
    for dc in range(DC):
        rg = act.tile([128, 512], BF16, tag="rg", bufs=2, name="lnrg")
        nc.vector.tensor_scalar_mul(out=rg[:], in0=rstd[:],
                                    scalar1=g_sb[:, ln_i, dc:dc + 1])
        am = act.tile([128, 512], BF16, tag="am", bufs=2, name="lnam")
        nc.vector.tensor_tensor(out=am[:], in0=a_t[:, dc, s0:s0 + 512],
                                in1=mean_sb[:], op=OP.subtract)
        nc.vector.tensor_tensor(out=am[:], in0=am[:], in1=rg[:],
                                op=OP.mult)
        # fp8 (x/8) first: the AllGather wire + matmuls consume this
        nc.vector.tensor_scalar(out=out8[:, dc, s0:s0 + 512], in0=am[:],
                                scalar1=b_sb[:, ln_i, dc:dc + 1],
                                scalar2=XS, op0=OP.add, op1=OP.mult)
        nc.gpsimd.tensor_scalar_add(out=out[:, dc, s0:s0 + 512], in0=am[:],
                                    scalar1=b_sb[:, ln_i, dc:dc + 1])
    if mean_out is not None:
        for dc in range(DC):
            nc.vector.tensor_reduce(out=mean_out[:, nq, dc, :],
                                    in_=out[:, dc, s0:s0 + 512],
                                    axis=mybir.AxisListType.X, op=OP.add)
    if post_half is not None:
        post_half(nq, out8)


def _emit_attn(nc, pools, dram, xq_bf, xq8, xkv8, li, bi, wqkv, bqo,
               is_cross=False):
    """One attention block. xq_bf [128,4,1024] bf16 residual; xq8/xkv8
    fp8 (1/8 scale). wqkv/bqo are prefetched weight tiles.
    Returns (x bf16, x8 fp8) post-LN."""
    act, pbig, pp, pctx, consts = (pools["act"], pools["pbig"], pools["pp"],
                                   pools["pctx"], pools["consts"])

    # ---- Q/K projections -> fp8 natural scale ----
    QT8 = act.tile([128, DC, 1024], F8, tag="qt")
    KT8 = act.tile([128, DC, 1024], F8, tag="kt")
    for pi, (dst, src8) in enumerate(((QT8, xq8), (KT8, xkv8))):
        for nq in range(NQ):
            c0 = nq * 512
            for mc in range(DC):
                ps = pp.tile([128, 512], F32, tag="pp")
                for hl in range(2):
                    for kcp in range(2):
                        nc.tensor.matmul(
                            ps[:],
                            wqkv[:, hl, pi * 4 + 2 * kcp:pi * 4 + 2 * kcp + 2,
                                 mc * 128:(mc + 1) * 128],
                            src8[:, 2 * kcp:2 * kcp + 2, c0:c0 + 512],
                            start=(hl == 0 and kcp == 0),
                            stop=(hl == 1 and kcp == 1), perf_mode=DR)
                if pi == 0:
                    # QT8 = ps/8 + bq  (Act: copy(scale*in)+bias; bq raw)
                    nc.scalar.activation(out=dst[:, mc, c0:c0 + 512],
                                         in_=ps, func=AF.Identity,
                                         scale=XS,
                                         bias=bqo[:, 0, mc:mc + 1])
                else:
                    nc.vector.tensor_scalar_mul(out=dst[:, mc, c0:c0 + 512],
                                                in0=ps, scalar1=XS)

    # ---- V projection (natural layout rows=positions) ----
    vext = []
    for stp in range(4):
        vt = pools["vext"].tile([128, 2, H, DK + 2], F8, tag=f"v{stp}",
                                name=f"vt{stp}")
        vext.append(vt)
    for st in range(NST):
        ps = pp.tile([128, 512], F32, tag="pp")
        for hl in range(2):
            for kcp in range(2):
                nc.tensor.matmul(
                    ps[:],
                    xkv8[:, 2 * kcp:2 * kcp + 2, st * 128:(st + 1) * 128],
                    wqkv[:, hl, 8 + 2 * kcp:8 + 2 * kcp + 2, :],
                    start=(hl == 0 and kcp == 0),
                    stop=(hl == 1 and kcp == 1), perf_mode=DR)
        vt = vext[st // 2]
        nc.vector.tensor_scalar_mul(
            out=vt[:, st % 2, :, 0:DK],
            in0=ps.rearrange("p (h k) -> p h k", h=H),
            scalar1=XS)
        nc.gpsimd.tensor_copy(out=vt[:, st % 2, :, DK:DK + 2],
                              in_=consts["c82"][:])

    # ---- attention core: scoresT (fp8 plain) -> exp -> ctx (DoubleRow) ----
    ctxT8 = act.tile([128, DC, 1024], F8, tag="ctxT")
    def head_scores(h, st_lo, st_hi, store):
        hp, hr = h // 2, (h % 2) * 64
        for st in range(st_lo, st_hi):
            sps = pbig.tile([128, 1024], F32, tag="big", name="sps")
            for nq in range(NQ):
                nc.tensor.matmul(sps[:, nq * 512:(nq + 1) * 512],
                                 KT8[hr:hr + 64, hp, st * 128:(st + 1) * 128],
                                 QT8[hr:hr + 64, hp, nq * 512:(nq + 1) * 512],
                                 start=True, stop=True)
            stp = st // 2
            if store.get(stp) is None:
                store[stp] = act.tile([128, 2, 1024], F8, tag=store["tag"](stp),
                                      bufs=1, name=f"e{h}_{stp}")
            nc.scalar.activation(out=store[stp][:, st % 2, :], in_=sps[:],
                                 func=AF.Exp,
                                 scale=float(1.0 / np.sqrt(DK)),
                                 bias=consts["expb_col"][:])

    def head_ctx(h, store):
        hp, hr = h // 2, (h % 2) * 64
        cpsA = pctx.tile([DK + 2, 512], F32, tag="ctx", name="cpsA")
        cpsB = pctx.tile([DK + 2, 512], F32, tag="ctx", name="cpsB")
        cps_nq = (cpsA, cpsB)
        for stp in range(4):
            for nq in range(NQ):
                nc.tensor.matmul(cps_nq[nq][:],
                                 vext[stp][:, :, h, :],
                                 store[stp][:, :, nq * 512:(nq + 1) * 512],
                                 start=(stp == 0), stop=(stp == 3),
                                 perf_mode=DR)
        rd = act.tile([1, 1024], BF16, tag="rd", bufs=2, name="rdl")
        rdb = act.tile([64, 1024], BF16, tag="rdb", bufs=2, name="rdbl")
        for nq in range(NQ):
            cc = nq * 512
            cps = cps_nq[nq]
            nc.vector.reciprocal(out=rd[:, cc:cc + 512],
                                 in_=cps[DK:DK + 1, :])
            nc.gpsimd.partition_broadcast(out_ap=rdb[:, cc:cc + 512],
                                          in_ap=rd[:, cc:cc + 512])
            nc.vector.tensor_tensor(out=ctxT8[hr:hr + 64, hp, cc:cc + 512],
                                    in0=cps[0:DK, :],
                                    in1=rdb[:, cc:cc + 512], op=OP.mult)

    if is_cross:
        # two-pass for heads 0-3: st0-3 exps run while AG chunk 1 is in
        # flight (stashed per-head), then st4-7 + ctx
        stores = {}
        for h in range(4):
            stores[h] = {"tag": lambda stp, h=h: f"stash{h}_{stp}"}
            head_scores(h, 0, 4, stores[h])
        for h in range(4):
            stores[h]["tag"] = lambda stp: f"exp{stp}"
            head_scores(h, 4, 8, stores[h])
            head_ctx(h, stores[h])
        for h in range(4, 8):
            store = {"tag": lambda stp: f"exp{stp}"}
            head_scores(h, 0, 8, store)
            head_ctx(h, store)
    else:
        for h in range(H):
            store = {"tag": lambda stp: f"exp{stp}"}
            head_scores(h, 0, 8, store)
            head_ctx(h, store)

    # ---- O projection + bias + residual, LN interleaved per half ----
    a_t = act.tile([128, DC, 1024], BF16, tag="a", bufs=1)
    out = pools["xpool"].tile([128, DC, 1024], BF16, tag="x", name="xout")
    out8 = pools["xpool"].tile([128, DC, 1024], F8, tag="x8", name="xout8")
    for nq in range(NQ):
        c0 = nq * 512
        for mc in range(DC):
            ps = pp.tile([128, 512], F32, tag="pp")
            for kcp in range(2):
                for hl in range(2):
                    nc.tensor.matmul(
                        ps[:],
                        wqkv[:, hl, 12 + 2 * kcp:12 + 2 * kcp + 2,
                             mc * 128:(mc + 1) * 128],
                        ctxT8[:, 2 * kcp:2 * kcp + 2, c0:c0 + 512],
                        start=(kcp == 0 and hl == 0),
                        stop=(kcp == 1 and hl == 1), perf_mode=DR)
            tmp = act.tile([128, 512], BF16, tag="otmp", bufs=2)
            # (ps + 8*bo)/8 = ctx@Wo + bo   (bo prescaled x8 on host)
            nc.vector.tensor_scalar(out=tmp[:], in0=ps[:],
                                    scalar1=bqo[:, 1, mc:mc + 1],
                                    scalar2=XS, op0=OP.add, op1=OP.mult)
            eng = nc.vector if mc % 2 == 0 else nc.gpsimd
            eng.tensor_tensor(out=a_t[:, mc, c0:c0 + 512], in0=tmp[:],
                              in1=xq_bf[:, mc, c0:c0 + 512], op=OP.add)
        _emit_ln_half(nc, pools, a_t, pools["lng_sb"][li],
                      pools["lnb_sb"][li], bi, nq, out, out8)
    return out, out8


def _emit_ffn(nc, pools, dram, x_bf, x8, li, post_half=None,
              prefetch_cb=None, mean_out=None):
    act, pp, consts = pools["act"], pools["pp"], pools["consts"]
    ones_512 = consts["ones_512"]
    w1 = pools["w"].tile([128, 2, 4, 2048], F8, tag="wbig")
    nc.sync.dma_start(w1[:], dram["ff_w1"][li])
    w2 = pools["w"].tile([128, 2, 16, 512], F8, tag="wbig")
    nc.sync.dma_start(w2[:], dram["ff_w2"][li])
    b1r = act.tile([1, 2048], BF16, tag="b1r", bufs=1)
    nc.sync.dma_start(b1r[:], dram["ff_b1r"][li])
    b2 = act.tile([128, 4], F32, tag="b2", bufs=2)
    nc.sync.dma_start(b2[:], dram["ff_b2"][li])
    if prefetch_cb is not None:
        prefetch_cb()

    a_t = act.tile([128, DC, 1024], BF16, tag="a", bufs=1)
    out = pools["xpool"].tile([128, DC, 1024], BF16, tag="x", name="fout")
    out8 = pools["xpool"].tile([128, DC, 1024], F8, tag="x8", name="fout8")
    for half in range(NQ):
        s0 = half * 512
        hT8 = act.tile([128, 16, 512], F8, tag="hT", bufs=2)
        for mf in range(FC):
            ps = pp.tile([128, 512], F32, tag="pp")
            # bias first (8*b1, bf16 ones moving), then hi/lo DoubleRow
            nc.tensor.matmul(ps[:], b1r[0:1, mf * 128:(mf + 1) * 128],
                             ones_512[:], start=True, stop=False)
            for hl in range(2):
                for kcp in range(2):
                    nc.tensor.matmul(
                        ps[:],
                        w1[:, hl, 2 * kcp:2 * kcp + 2,
                           mf * 128:(mf + 1) * 128],
                        x8[:, 2 * kcp:2 * kcp + 2, s0:s0 + 512],
                        start=False, stop=(hl == 1 and kcp == 1),
                        perf_mode=DR)
            # h/8 = relu(ps/64)
            nc.scalar.activation(out=hT8[:, mf, :], in_=ps[:],
                                 func=AF.Relu, scale=1.0 / 64.0)
        for mc in range(DC):
            ps = pp.tile([128, 512], F32, tag="pp")
            for kfp in range(8):
                for hl in range(2):
                    nc.tensor.matmul(
                        ps[:],
                        w2[:, hl, 2 * kfp:2 * kfp + 2,
                           mc * 128:(mc + 1) * 128],
                        hT8[:, 2 * kfp:2 * kfp + 2, :],
                        start=(kfp == 0 and hl == 0),
                        stop=(kfp == 7 and hl == 1), perf_mode=DR)
            tmp = act.tile([128, 512], BF16, tag="ftmp", bufs=2)
            # (ps + 8*b2)/8 = h@W2 + b2   (b2 prescaled x8 on host)
            nc.vector.tensor_scalar(out=tmp[:], in0=ps[:],
                                    scalar1=b2[:, mc:mc + 1],
                                    scalar2=XS, op0=OP.add, op1=OP.mult)
            eng = nc.vector if mc % 2 == 0 else nc.gpsimd
            eng.tensor_tensor(out=a_t[:, mc, s0:s0 + 512], in0=tmp[:],
                              in1=x_bf[:, mc, s0:s0 + 512], op=OP.add)
        _emit_ln_half(nc, pools, a_t, pools["lng_sb"][li],
                      pools["lnb_sb"][li], 2, half, out, out8,
                      mean_out=mean_out, post_half=post_half)
    return out, out8


def _build(n_layers=LAYERS):
    nc = bacc.Bacc("TRN2", target_bir_lowering=False, debug=False,
                   num_devices=NCORES)

    dram = {}
    # both channels' inputs/embedding weights (partner layer-0 computed
    # locally)
    dram["wT"] = nc.dram_tensor("wT", [2, IN, S], BF16, kind="ExternalInput")
    dram["w_in"] = nc.dram_tensor("w_in", [2, IN, D], BF16,
                                  kind="ExternalInput")
    dram["b_in"] = nc.dram_tensor("b_in", [2, 128, DC], F32,
                                  kind="ExternalInput")
    dram["peT"] = nc.dram_tensor("peT", [128, DC, S], BF16,
                                 kind="ExternalInput")
    dram["qkv_w"] = nc.dram_tensor("qkv_w", [LAYERS, 2, 128, 2, 16, 512], F8,
                                   kind="ExternalInput")
    dram["qo_b"] = nc.dram_tensor("qo_b", [LAYERS, 2, 128, 2, 4], F32,
                                  kind="ExternalInput")
    dram["ln_g"] = nc.dram_tensor("ln_g", [128, LAYERS, 3, 4], F32,
                                  kind="ExternalInput")
    dram["ln_b"] = nc.dram_tensor("ln_b", [128, LAYERS, 3, 4], F32,
                                  kind="ExternalInput")
    dram["ff_w1"] = nc.dram_tensor("ff_w1", [LAYERS, 128, 2, 4, 2048], F8,
                                   kind="ExternalInput")
    dram["ff_b1r"] = nc.dram_tensor("ff_b1r", [LAYERS, 1, 2048], BF16,
                                    kind="ExternalInput")
    dram["ff_w2"] = nc.dram_tensor("ff_w2", [LAYERS, 128, 2, 16, 512], F8,
                                   kind="ExternalInput")
    dram["ff_b2"] = nc.dram_tensor("ff_b2", [LAYERS, 128, 4], F32,
                                   kind="ExternalInput")
    dram["hd_w1"] = nc.dram_tensor("hd_w1", [2, 128, 8, 512], BF16,
                                   kind="ExternalInput")
    dram["hd_b1"] = nc.dram_tensor("hd_b1", [2, 128, 4], F32,
                                   kind="ExternalInput")
    dram["hd_w2"] = nc.dram_tensor("hd_w2", [2, 128, 4, 2], BF16,
                                   kind="ExternalInput")
    dram["hd_b2"] = nc.dram_tensor("hd_b2", [1, 2, 2], F32,
                                   kind="ExternalInput")
    out_logits = nc.dram_tensor("logits", [1, 4], F32, kind="ExternalOutput")

    rg_pairs = [[0, 1], [2, 3], [4, 5], [6, 7]]

    with tile.TileContext(nc) as tc:
        with (
            nc.allow_low_precision(
                reason="deliberate fp8/bf16 activation pipeline"),
            tc.tile_pool(name="act", bufs=1) as act,
            tc.tile_pool(name="w", bufs=2) as wpool,
            tc.tile_pool(name="vext", bufs=1) as vpool,
            tc.tile_pool(name="consts", bufs=1) as cpool,
            tc.tile_pool(name="x", bufs=3) as xpool,
            tc.tile_pool(name="pbig", bufs=2, space="PSUM") as pbig,
            tc.tile_pool(name="pp", bufs=2, space="PSUM") as pp,
            tc.tile_pool(name="pctx", bufs=2, space="PSUM") as pctx,
            tc.tile_pool(name="dram", bufs=1, space="DRAM") as dpool,
        ):
            # ---- constants ----
            oavg_bf = cpool.tile([128, 128], BF16, tag="oavg_bf")
            nc.vector.memset(oavg_bf[:], 1.0 / D)
            eps_col = cpool.tile([128, 1], F32, tag="eps_col")
            nc.vector.memset(eps_col[:], EPS)
            expb_col = cpool.tile([128, 1], F32, tag="expb_col")
            nc.vector.memset(expb_col[:], EB)
            ones_512 = cpool.tile([1, 512], BF16, tag="ones512")
            nc.vector.memset(ones_512[:], 1.0)
            c82 = cpool.tile([128, 8, 2], BF16, tag="c82")
            nc.vector.memset(c82[:, :, 0:1], 8.0)
            nc.vector.memset(c82[:, :, 1:2], 0.0)
            lng_sb = cpool.tile([128, LAYERS, 3, 4], F32, tag="lng")
            nc.sync.dma_start(lng_sb[:], dram["ln_g"][:])
            lnb_sb = cpool.tile([128, LAYERS, 3, 4], F32, tag="lnb")
            nc.sync.dma_start(lnb_sb[:], dram["ln_b"][:])
            lnb8_sb = cpool.tile([128, LAYERS, 3, 4], F32, tag="lnb8")
            nc.vector.tensor_scalar_mul(out=lnb8_sb[:], in0=lnb_sb[:],
                                        scalar1=XS)
            consts = dict(oavg_bf=oavg_bf, eps_col=eps_col,
                          expb_col=expb_col, ones_512=ones_512, c82=c82)
            pools = dict(act=act, w=wpool, vext=vpool, consts=consts,
                         pbig=pbig, pp=pp, pctx=pctx, xpool=xpool,
                         lng_sb=[lng_sb[:, li] for li in range(LAYERS)],
                         lnb_sb=[lnb_sb[:, li] for li in range(LAYERS)],
                         lnb8_sb=[lnb8_sb[:, li] for li in range(LAYERS)])

            # ---- layer 0 inputs: own channel (bf16+fp8) + partner (fp8) ----
            peT_sb = xpool.tile([128, DC, S], BF16, tag="x")
            nc.sync.dma_start(peT_sb[:], dram["peT"][:])
            wT_sb = act.tile([IN, 2, S], BF16, tag="wT")
            win_sb = act.tile([IN, 2, D], BF16, tag="win")
            bin_sb = act.tile([128, 2, DC], F32, tag="bin")
            for ch in range(2):
                nc.sync.dma_start(wT_sb[:, ch, :], dram["wT"][ch])
                nc.sync.dma_start(win_sb[:, ch, :], dram["w_in"][ch])
                nc.sync.dma_start(bin_sb[:, ch, :], dram["b_in"][ch])

            x_own = xpool.tile([128, DC, 1024], BF16, tag="x")
            x8_own = xpool.tile([128, DC, 1024], F8, tag="x8")
            x8_part = act.tile([128, DC, 1024], F8, tag="xpart0")
            for ch in range(2):  # 0 = own, 1 = partner
                for mc in range(DC):
                    ps = pbig.tile([128, 1024], F32, tag="big")
                    for nq in range(NQ):
                        nc.tensor.matmul(
                            ps[:, nq * 512:(nq + 1) * 512],
                            win_sb[:, ch, mc * 128:(mc + 1) * 128],
                            wT_sb[:, ch, nq * 512:(nq + 1) * 512],
                            start=True, stop=True)
                    if ch == 0:
                        nc.vector.scalar_tensor_tensor(
                            out=x_own[:, mc, :], in0=ps[:],
                            scalar=bin_sb[:, 0, mc:mc + 1],
                            in1=peT_sb[:, mc, :], op0=OP.add, op1=OP.add)
                        nc.vector.tensor_scalar_mul(
                            out=x8_own[:, mc, :], in0=x_own[:, mc, :],
                            scalar1=XS)
                    else:
                        for nq in range(NQ):
                            c0 = nq * 512
                            tmp = act.tile([128, 512], BF16, tag="otmp",
                                           bufs=2, name="x0ptmp")
                            nc.vector.scalar_tensor_tensor(
                                out=tmp[:], in0=ps[:, c0:c0 + 512],
                                scalar=bin_sb[:, 1, mc:mc + 1],
                                in1=peT_sb[:, mc, c0:c0 + 512],
                                op0=OP.add, op1=OP.add)
                            nc.gpsimd.tensor_scalar_mul(
                                out=x8_part[:, mc, c0:c0 + 512], in0=tmp[:],
                                scalar1=XS)

            pid = nc.sync.partition_id()
            partner_par = 1 - (pid % 2)

            def load_attn_w(li, bi):
                w = pools["w"].tile([128, 2, 16, 512], F8, tag="wbig",
                                    name=f"wqkv{li}_{bi}")
                nc.sync.dma_start(w[:], dram["qkv_w"][li % LAYERS, bi])
                b = act.tile([128, 2, 4], F32, tag="bqo", bufs=4,
                             name=f"bqo{li}_{bi}")
                nc.sync.dma_start(b[:], dram["qo_b"][li % LAYERS, bi])
                return w, b

            wnext = [load_attn_w(0, 0), load_attn_w(0, 1)]
            mean_halves = act.tile([128, NQ, DC, 1], F32, tag="meanh")
            hd_tiles = []
            for hd in range(2):
                hw1 = act.tile([128, 4, 512], BF16, tag="hw1", bufs=4,
                               name=f"hw1_{hd}")
                nc.sync.dma_start(hw1[:], dram["hd_w1"][hd, :, 0:4, :])
                hw1b = act.tile([128, 4, 512], BF16, tag="hw1", bufs=4,
                                name=f"hw1b_{hd}")
                nc.sync.dma_start(hw1b[:], dram["hd_w1"][hd, :, 4:8, :])
                hw2 = act.tile([128, 4, 2], BF16, tag="hw2", bufs=2,
                               name=f"hw2_{hd}")
                nc.sync.dma_start(hw2[:], dram["hd_w2"][hd])
                hb1 = act.tile([128, 4], F32, tag="hb1", bufs=2,
                               name=f"hb1_{hd}")
                nc.sync.dma_start(hb1[:], dram["hd_b1"][hd])
                hd_tiles.append((hw1, hw1b, hw2, hb1))

            def make_post_half(li):
                if li >= n_layers - 1:
                    return None

                outs = {}

                def post_half(nq, out8):
                    s0 = nq * 512
                    ag_in = dpool.tile([128, DC, 512], F8,
                                       tag=f"agin{li}_{nq}")
                    ag_out = dpool.tile([2, 128, DC, 512], F8,
                                        tag=f"agout{li}_{nq}")
                    nc.sync.dma_start(ag_in[:], out8[:, :, s0:s0 + 512])
                    nc.gpsimd.collective_compute(
                        "AllGather", OP.bypass, replica_groups=rg_pairs,
                        ins=[ag_in.opt()], outs=[ag_out.opt()])
                    outs[nq] = ag_out
                    if nq == NQ - 1:
                        for k in range(NQ):
                            nc.sync.dma_start(
                                nxt_x8_part[:, :, k * 512:(k + 1) * 512],
                                outs[k][ds(partner_par, 1), :, :, :].opt())
                return post_half

            for li in range(n_layers):
                lw = li % LAYERS
                if li < n_layers - 1:
                    nxt_x8_part = act.tile([128, DC, 1024], F8,
                                           tag=f"xpart{1 - li % 2}")
                (wA, bA), (wB, bB) = wnext
                xc_bf, xc8 = _emit_attn(nc, pools, dram, x_own, x8_own,
                                        x8_part, lw, 0, wA, bA,
                                        is_cross=True)
                xs_bf, xs8 = _emit_attn(nc, pools, dram, xc_bf, xc8, xc8,
                                        lw, 1, wB, bB)

                def prefetch(li=li):
                    if li < n_layers - 1:
                        wnext[0] = load_attn_w(li + 1, 0)
                        wnext[1] = load_attn_w(li + 1, 1)

                mo = mean_halves if li == n_layers - 1 else None
                x_own, x8_own = _emit_ffn(nc, pools, dram, xs_bf, xs8, lw,
                                          post_half=make_post_half(li),
                                          prefetch_cb=prefetch, mean_out=mo)
                if li < n_layers - 1:
                    x8_part = nxt_x8_part

            # ---- mean pool over S -> pairwise allgather -> heads ----
            mean_sb = act.tile([128, DC, 1], F32, tag="mean")
            nc.vector.tensor_tensor(out=mean_sb[:], in0=mean_halves[:, 0],
                                    in1=mean_halves[:, 1], op=OP.add)
            mb_in = dpool.tile([DC, 128, 1], F32, tag="mbin")
            nc.sync.dma_start(mb_in.rearrange("d p o -> p d o"),
                              mean_sb[:])
            mb_out = dpool.tile([2 * DC, 128, 1], F32, tag="mbout")
            nc.gpsimd.collective_compute(
                "AllGather", OP.bypass, replica_groups=rg_pairs,
                ins=[mb_in.opt()], outs=[mb_out.opt()])
            fusedT = act.tile([128, 2 * DC, 1], F32, tag="fusedT")
            nc.sync.dma_start(fusedT[:],
                              mb_out.rearrange("d p o -> p d o"))
            fusedb = act.tile([128, 2 * DC, 1], BF16, tag="fusedb")
            nc.vector.tensor_copy(out=fusedb[:], in_=fusedT[:])

            hb2 = act.tile([1, 2, 2], F32, tag="hb2")
            nc.sync.dma_start(hb2[:], dram["hd_b2"][:])
            logits_sb = act.tile([1, 4], F32, tag="logits")
            for hd in range(2):
                hw1 = hd_tiles[hd][0]
                hw1b = hd_tiles[hd][1]
                hw2 = hd_tiles[hd][2]
                hb1 = hd_tiles[hd][3]
                o1 = act.tile([128, 4, 1], BF16, tag="o1", bufs=2)
                for mc in range(DC):
                    ps = pbig.tile([128, 1024], F32, tag="big")
                    for kc in range(2 * DC):
                        hw = hw1 if kc < 4 else hw1b
                        nc.tensor.matmul(
                            ps[:, 0:1],
                            hw[:, kc % 4, mc * 128:(mc + 1) * 128],
                            fusedb[:, kc, :],
                            start=(kc == 0), stop=(kc == 2 * DC - 1))
                    nc.vector.tensor_scalar(out=o1[:, mc, :], in0=ps[:, 0:1],
                                            scalar1=hb1[:, mc:mc + 1],
                                            scalar2=0.0, op0=OP.add,
                                            op1=OP.max)
                lp = pbig.tile([128, 1024], F32, tag="big")
                for kc in range(DC):
                    nc.tensor.matmul(lp[0:1, 0:2], o1[:, kc, :],
                                     hw2[:, kc, :],
                                     start=(kc == 0), stop=(kc == DC - 1))
                nc.vector.tensor_tensor(out=logits_sb[0:1, hd * 2:hd * 2 + 2],
                                        in0=lp[0:1, 0:2], in1=hb2[0:1, hd, :],
                                        op=OP.add)
            nc.sync.dma_start(out_logits[:], logits_sb[:])

    nc.compile()
    return nc


def _hilo(w):
    """Split w into fp8 hi/lo at 64x scale; returns [2, ...] fp8 array."""
    ws = (np.asarray(w, np.float32) * WS)
    hi = ws.astype(F8NP)
    lo = (ws - hi.astype(np.float32)).astype(F8NP)
    return np.stack([hi, lo])


def _prep(inputs):
    f32 = np.float32

    def g(k):
        return np.asarray(inputs[k], f32)

    lw, rw = g("left_wrist"), g("right_wrist")
    Wl, bl, Wr, br, pe = g("Wl"), g("bl"), g("Wr"), g("br"), g("pe")
    mha_w, mha_b = g("mha_w"), g("mha_b")
    mha_ln_g, mha_ln_b = g("mha_ln_g"), g("mha_ln_b")
    ff_w1, ff_b1, ff_w2, ff_b2 = g("ff_w1"), g("ff_b1"), g("ff_w2"), g("ff_b2")
    ff_ln_g, ff_ln_b = g("ff_ln_g"), g("ff_ln_b")
    h_w1 = [g("h1_w1"), g("h2_w1")]
    h_b1 = [g("h1_b1"), g("h2_b1")]
    h_w2 = [g("h1_w2"), g("h2_w2")]
    h_b2 = [g("h1_b2"), g("h2_b2")]

    peT = np.ascontiguousarray(
        pe.T.reshape(DC, 128, S).transpose(1, 0, 2)).astype(BF)

    per_ch = {}
    for ch in range(2):
        blocks = (0, 2) if ch == 0 else (1, 3)
        qkv = np.zeros((LAYERS, 2, 128, 2, 16, 512), F8NP)
        qob = np.zeros((LAYERS, 2, 128, 2, 4), f32)
        lng = np.zeros((128, LAYERS, 3, 4), f32)
        lnb = np.zeros((128, LAYERS, 3, 4), f32)
        fw1 = np.zeros((LAYERS, 128, 2, 4, 2048), F8NP)
        fb1r = np.zeros((LAYERS, 1, 2048), BF)
        fw2 = np.zeros((LAYERS, 128, 2, 16, 512), F8NP)
        fb2 = np.zeros((LAYERS, 128, 4), f32)
        for li in range(LAYERS):
            for bi, blk in enumerate(blocks):
                for pi in range(4):  # q, k, v, o
                    wt = mha_w[li, blk, pi].reshape(DC, 128, D) \
                        .transpose(1, 0, 2)          # [128, DC, D]
                    hl = _hilo(wt)                   # [2, 128, DC, D]
                    qkv[li, bi, :, :, pi * 4:(pi + 1) * 4, :] = \
                        hl.transpose(1, 0, 2, 3)
                # q bias (x8) ; o bias + v_b @ W_o (x8)
                qob[li, bi, :, 0, :] = \
                    mha_b[li, blk, 0].reshape(DC, 128).T
                ob = mha_b[li, blk, 3] + mha_b[li, blk, 2] @ mha_w[li, blk, 3]
                qob[li, bi, :, 1, :] = (8.0 * ob).reshape(DC, 128).T
                lng[:, li, bi, :] = mha_ln_g[li, blk].reshape(DC, 128).T
                lnb[:, li, bi, :] = mha_ln_b[li, blk].reshape(DC, 128).T
            lng[:, li, 2, :] = ff_ln_g[li, ch].reshape(DC, 128).T
            lnb[:, li, 2, :] = ff_ln_b[li, ch].reshape(DC, 128).T
            w1t = ff_w1[li, ch].reshape(DC, 128, F).transpose(1, 0, 2)
            fw1[li] = _hilo(w1t).transpose(1, 0, 2, 3)
            fb1r[li, 0] = (8.0 * ff_b1[li, ch]).astype(BF)
            w2t = ff_w2[li, ch].reshape(FC, 128, D).transpose(1, 0, 2)
            fw2[li] = _hilo(w2t).transpose(1, 0, 2, 3)
            fb2[li] = (8.0 * ff_b2[li, ch]).reshape(DC, 128).T
        per_ch[ch] = dict(qkv_w=qkv, qo_b=qob, ln_g=lng, ln_b=lnb,
                          ff_w1=fw1, ff_b1r=fb1r, ff_w2=fw2, ff_b2=fb2)

    hd_w1 = np.stack([(w / float(S)).reshape(2 * DC, 128, D)
                      .transpose(1, 0, 2) for w in h_w1]).astype(BF)
    hd_b1 = np.stack([b.reshape(DC, 128).T for b in h_b1]).astype(f32)
    hd_w2 = np.stack([w.reshape(DC, 128, 2).transpose(1, 0, 2)
                      for w in h_w2]).astype(BF)
    hd_b2 = np.stack([b.reshape(1, 2) for b in h_b2]) \
        .transpose(1, 0, 2).astype(f32)

    in_maps = []
    for core in range(NCORES):
        b, ch = core // 2, core % 2
        wrists = [lw[b], rw[b]] if ch == 0 else [rw[b], lw[b]]
        w_ins = [Wl, Wr] if ch == 0 else [Wr, Wl]
        b_ins = [bl, br] if ch == 0 else [br, bl]
        m = {k: np.ascontiguousarray(v) for k, v in per_ch[ch].items()}
        m["wT"] = np.ascontiguousarray(
            np.stack([w.T for w in wrists])).astype(BF)
        m["w_in"] = np.ascontiguousarray(np.stack(w_ins)).astype(BF)
        m["b_in"] = np.ascontiguousarray(
            np.stack([bb.reshape(DC, 128).T for bb in b_ins]).astype(f32))
        m["peT"] = peT
        m["hd_w1"] = hd_w1
        m["hd_b1"] = hd_b1
        m["hd_w2"] = hd_w2
        m["hd_b2"] = hd_b2
        in_maps.append(m)
    return in_maps


def run(inputs, trace=False, n_layers=LAYERS):
    key = ("nc", n_layers)
    if key not in _CACHE:
        _CACHE[key] = _build(n_layers)
    nc = _CACHE[key]
    in_maps = _prep(inputs)
    res = run_bass_kernel_spmd(nc, in_maps, core_ids=list(range(NCORES)),
                               trace=trace)
    logits1 = np.zeros((B, 2), np.float32)
    logits2 = np.zeros((B, 2), np.float32)
    for b in range(B):
        out = res.results[2 * b]["logits"]
        logits1[b] = out[0, 0:2]
        logits2[b] = out[0, 2:4]
    return (logits1, logits2), res


def kernel(**inputs):
    out, _ = run(inputs, trace=False)
    return out
